# revision 36
# baseline (speedup 1.0000x reference)
"""Trainium2 Bass kernel for the SE + patch-correlation-attention + down-conv module.

Sharding (8 cores): split the 96 image rows into 8 slabs of 12 rows.

Execution-time model (measured): the FIRST collective of every NEFF execution
completes ~90us after exec start regardless of issue time; later collectives
take ~10us. So a dummy warm-up AllReduce is issued at t=0 and the entire
attention pipeline (which has no cross-core dependency once the SE gate is
known) runs hidden under that window. The SE gate is computed on-device from a
host-precomputed global channel mean (the only cross-core quantity), so no
collective is needed before the AllToAll.

Per core:
  1. warm-up AllReduce (absorbs the ~90us collective-path init)
  2. SE gate y from host-shipped channel mean (tiny MLP on device)
  3. FF = x*y, S = sigmoid(x*y) on the 14-row halo slab (bf16)
  4. patch correlation (9 shifts, bf16 DVE products + PE one-hot reduction),
     softmax (pre-normalized rows), weighted sum -> attention out (256 x 1152)
  5. the a2a staging write scrambles to (dest, b, s, t) layout on the SENDER
     (descriptor-heavy, but hidden pre-a2a) so the post-a2a rhs loads are
     contiguous
  6. AllToAll (bf16), then contiguous rhs loads, 256x256 down matmul,
     InstanceNorm partials -> tiny AllReduce, normalize + LeakyReLU,
     write the (256, 32, 36) bf16 output slice
Host: computes the channel mean, shards x (bf16), gathers + permutes output.

ScalarE activation-table loads (~2.7us each) are prefetched with dummy 1-elem
activations so no table switch lands on the critical path.
"""
import numpy as np

C, H, W, M = 256, 96, 96, 8
RPC = H // M          # 12 rows per core
P = RPC * W           # 1152 positions per core
SLAB = RPC + 2        # 14 rows incl. halo
WP = 100              # padded slab width (j0 at col 2)
HW = H * W            # 9216
SHIFTS = [(di, dj) for di in (-1, 0, 1) for dj in (-1, 0, 1)]
CHUNKS = [(0, 512), (512, 512), (1024, 128)]   # psum-bank-aligned matmul N-chunks

_cache = {}


def _build():
    import concourse.bass as bass
    from concourse import bacc
    import concourse.mybir as mybir
    from concourse.tile import TileContext
    from concourse.masks import make_identity

    fp32 = mybir.dt.float32
    bf16 = mybir.dt.bfloat16
    AF = mybir.ActivationFunctionType
    Alu = mybir.AluOpType
    GROUPS = [list(range(M))]

    nc = bacc.Bacc()

    xs = nc.declare_dram_parameter("xs", [C, SLAB, W], bf16, isOutput=False)
    mn = nc.declare_dram_parameter("mn", [128, 2], fp32, isOutput=False)
    msk = nc.declare_dram_parameter("msk", [128, 2], fp32, isOutput=False)
    w1t = nc.declare_dram_parameter("w1t", [C, 16], fp32, isOutput=False)
    b1 = nc.declare_dram_parameter("b1", [16, 1], fp32, isOutput=False)
    w2t = nc.declare_dram_parameter("w2t", [16, C], fp32, isOutput=False)
    b2 = nc.declare_dram_parameter("b2", [C, 1], fp32, isOutput=False)
    dwt = nc.declare_dram_parameter("dwt", [C, C], bf16, isOutput=False)
    outp = nc.declare_dram_parameter("out", [C, 32, 36], bf16, isOutput=True)

    warm_in = nc.dram_tensor("warm_in", [1, 1], fp32)
    warm_out = nc.dram_tensor("warm_out", [1, 1], fp32, addr_space="Shared")
    # a2a chunks in (dest, s, p) layout: s = attention channel within the
    # dest's 32-slice, p = sender-local position; the receiver de-scrambles
    # with strided DRAM reads (cheaper than write-side scrambling)
    a2a_in = nc.dram_tensor("a2a_in", [M, 32, P], bf16)
    a2a_out = nc.dram_tensor("a2a_out", [M, 32, P], bf16)
    st_part = nc.dram_tensor("st_part", [128, 4], fp32)
    st_sum = nc.dram_tensor("st_sum", [128, 4], fp32, addr_space="Shared")
    a_dram = nc.dram_tensor("a_dram", [9, P], bf16)
    r_dram = nc.dram_tensor("r_dram", [P], fp32)
    dma = nc.default_dma_engine

    with TileContext(nc) as tc:
        with (
            tc.tile_pool(name="const", bufs=1) as cp,
            tc.tile_pool(name="sb", bufs=1) as sp,
            tc.tile_pool(name="work", bufs=6) as wp,
        ):
            # ---------- small consts first (y-gate path), then x slabs ------
            mn_sb = cp.tile([128, 2], fp32, tag="mn")
            dma.dma_start(out=mn_sb, in_=mn[:, :])
            b1_sb = cp.tile([16, 1], fp32)
            nc.scalar.dma_start(out=b1_sb, in_=b1[:, :])
            b2_sb = cp.tile([128, 2], fp32)
            w1_sb = [cp.tile([128, 16], fp32, tag=f"w1_{ct}", name=f"w1_{ct}") for ct in range(2)]
            for ct in range(2):
                nc.scalar.dma_start(out=b2_sb[:, ct : ct + 1], in_=b2[128 * ct : 128 * ct + 128, :])
                dma.dma_start(out=w1_sb[ct], in_=w1t[128 * ct : 128 * ct + 128, :])
            w2_sb = cp.tile([16, C], fp32)
            dma.dma_start(out=w2_sb, in_=w2t[:, :])

            x_sb = [sp.tile([128, SLAB, W], bf16, tag=f"x{ct}", name=f"x{ct}") for ct in range(2)]
            dma.dma_start(out=x_sb[0], in_=xs[0:128, :, :])
            nc.scalar.dma_start(out=x_sb[1], in_=xs[128:256, :, :])

            msk_sb = cp.tile([128, 2], fp32)
            nc.gpsimd.dma_start(out=msk_sb, in_=msk[:, :])
            dw_sb = [cp.tile([128, C], bf16, tag=f"dw_{ct}", name=f"dw_{ct}") for ct in range(2)]
            for ct in range(2):
                nc.gpsimd.dma_start(out=dw_sb[ct], in_=dwt[128 * ct : 128 * ct + 128, :])

            # ---------- constants ----------
            ident = cp.tile([128, 128], bf16)
            make_identity(nc, ident)
            e_all = cp.tile([128, 9, 9], bf16)
            nc.gpsimd.memset(e_all, 0.0)
            for d in range(9):
                nc.gpsimd.memset(e_all[:, d, d : d + 1], 1.0)
            ones_99 = cp.tile([9, 9], bf16)
            nc.gpsimd.memset(ones_99, 1.0)
            eps_sb = cp.tile([128, 1], fp32)
            nc.gpsimd.memset(eps_sb, 1e-5)
            dm1 = cp.tile([1, 1], fp32)
            dm2 = cp.tile([1, 1], fp32)
            nc.vector.memset(dm1, 0.0)
            nc.vector.memset(dm2, 0.0)

            # ---------- SE gate from host-shipped channel mean ----------
            with tc.tile_pool(name="ps_se", bufs=1, space="PSUM") as pse:
                h_ps = pse.tile([16, 1], fp32)
                for ct in range(2):
                    nc.tensor.matmul(
                        h_ps, w1_sb[ct], mn_sb[:, ct : ct + 1],
                        start=(ct == 0), stop=(ct == 1),
                    )
                h_sb = sp.tile([16, 1], fp32)
                # relu on DVE (avoids burning a ScalarE table slot on Relu)
                nc.vector.tensor_scalar(
                    out=h_sb, in0=h_ps, scalar1=b1_sb[:, 0:1], scalar2=0.0,
                    op0=Alu.add, op1=Alu.max,
                )
                y_ps = pse.tile([128, 2], fp32)
                y_sb = sp.tile([128, 2], fp32, tag="ygate")
                for ct in range(2):
                    nc.tensor.matmul(
                        y_ps[:, ct : ct + 1], w2_sb[:, 128 * ct : 128 * ct + 128], h_sb,
                        start=True, stop=True,
                    )
                    # first ScalarE op in the program: loads the sigmoid table
                    # set while the x slabs are still streaming in
                    nc.scalar.activation(out=y_sb[:, ct : ct + 1], in_=y_ps[:, ct : ct + 1],
                                         func=AF.Sigmoid, bias=b2_sb[:, ct : ct + 1], scale=1.0)

            # ---------- FF and S maps (bf16, zero-padded 14x100 layout) ------
            ff_sb = [sp.tile([128, SLAB, WP], bf16, tag=f"ff{ct}", name=f"ff{ct}") for ct in range(2)]
            s_sb = [sp.tile([128, SLAB, WP], bf16, tag=f"s{ct}", name=f"s{ct}") for ct in range(2)]
            ff2_sb = [sp.tile([128, SLAB, WP], bf16, tag=f"ff2{ct}", name=f"ff2{ct}") for ct in range(2)]
            s2_sb = [sp.tile([128, SLAB, WP], bf16, tag=f"s2{ct}", name=f"s2{ct}") for ct in range(2)]
            for ct in range(2):
                nc.vector.memset(s2_sb[ct][:, :, 2:3], 0.0)
                nc.vector.memset(s2_sb[ct][:, :, 99:100], 0.0)
                nc.vector.memset(ff2_sb[ct][:, :, 2:3], 0.0)
                nc.vector.memset(ff2_sb[ct][:, :, 99:100], 0.0)
                nc.vector.tensor_scalar(
                    out=ff_sb[ct][:, :, 2:98], in0=x_sb[ct],
                    scalar1=y_sb[:, ct : ct + 1], scalar2=None, op0=Alu.mult,
                )
                nc.scalar.activation(
                    out=s_sb[ct][:, :, 2:98], in_=x_sb[ct],
                    func=AF.Sigmoid, scale=y_sb[:, ct : ct + 1],
                )
                nc.vector.tensor_scalar(
                    out=s_sb[ct][:, 0, 2:98], in0=s_sb[ct][:, 0, 2:98],
                    scalar1=msk_sb[:, 0:1], scalar2=None, op0=Alu.mult,
                )
                nc.vector.tensor_scalar(
                    out=s_sb[ct][:, 13, 2:98], in0=s_sb[ct][:, 13, 2:98],
                    scalar1=msk_sb[:, 1:2], scalar2=None, op0=Alu.mult,
                )
                nc.vector.tensor_copy(out=s2_sb[ct][:, :, 3:99], in_=s_sb[ct][:, :, 2:98])
                nc.vector.tensor_copy(out=ff2_sb[ct][:, :, 3:99], in_=ff_sb[ct][:, :, 2:98])

            # preload the exp table set while phase 2 runs; the input is a
            # slice of the ct1 S map purely to pin this op's position in the
            # ScalarE queue (after the S sigmoids, before the real exp)
            nc.scalar.activation(out=dm1, in_=s_sb[1][0:1, 0, 2:3], func=AF.Exp)

            def sview(ct, di, dj):
                if dj == 0:
                    return s_sb[ct][:, 1 + di : 13 + di, 2:98]
                return s2_sb[ct][:, 1 + di : 13 + di, 3 + dj : 99 + dj]

            def ffview(ct, di, dj):
                if dj == 0:
                    return ff_sb[ct][:, 1 + di : 13 + di, 2:98]
                return ff2_sb[ct][:, 1 + di : 13 + di, 3 + dj : 99 + dj]

            # ---------- phase 2: correlation  A[d, p] = sum_c S*S_d ----------
            with tc.tile_pool(name="ps_corr", bufs=1, space="PSUM") as pc:
                A_ps = pc.tile([9, P], fp32)
                for d, (di, dj) in enumerate(SHIFTS):
                    for ct in range(2):
                        # all products on DVE: GpSimd is ~3x slower per op and
                        # pays a ~4us program-load penalty on its first use,
                        # which made it the phase-2 tail when offloaded here
                        veng = nc.vector
                        prod = wp.tile([128, P], bf16, tag="prod")
                        pv = prod.rearrange("c (r w) -> c r w", w=W)
                        veng.tensor_tensor(
                            out=pv,
                            in0=s_sb[ct][:, 1:13, 2:98],
                            in1=sview(ct, di, dj),
                            op=Alu.mult,
                        )
                        for (o, n) in CHUNKS:
                            nc.tensor.matmul(
                                A_ps[:, o : o + n], e_all[:, d, :], prod[:, o : o + n],
                                start=(d == 0 and ct == 0), stop=(d == 8 and ct == 1),
                            )
                # softmax numerator: exp with the folded 1/C mean.  The
                # 1/denominator is deferred to after the phase-3 psum
                # accumulation so the row broadcasts start immediately.
                exp_sb = sp.tile([9, P], bf16, tag="exps")
                nc.scalar.activation(out=exp_sb, in_=A_ps, func=AF.Exp, scale=1.0 / C)
                dma.dma_start(out=a_dram[:, :], in_=exp_sb)
                den_ps = pc.tile([9, P], fp32)
                for (o, n) in CHUNKS:
                    nc.tensor.matmul(den_ps[:, o : o + n], ones_99, exp_sb[:, o : o + n],
                                     start=True, stop=True)
                rec_sb = sp.tile([9, P], fp32, tag="recs")
                nc.vector.reciprocal_approx_fast(out=rec_sb, in_=den_ps)

            # ---------- replicate exp rows + recip across 128 partitions ----
            rep_sb = [sp.tile([128, P], bf16, tag=f"rep{d}", name=f"rep{d}") for d in range(9)]
            for d in range(9):
                eng = dma if d % 2 == 0 else nc.scalar
                eng.dma_start(
                    out=rep_sb[d],
                    in_=a_dram[d, :].partition_broadcast(128),
                )
            rec_row = sp.tile([1, P], fp32, tag="recrow")
            nc.vector.tensor_copy(out=rec_row, in_=rec_sb[0:1, :])
            nc.scalar.dma_start(out=r_dram[:], in_=rec_row)
            rec_rep = sp.tile([128, P], fp32, tag="recrep")
            nc.scalar.dma_start(out=rec_rep, in_=r_dram[:].partition_broadcast(128))
            # preload the sqrt/prelu table set during phase 3 / a2a; input
            # slice of exp_sb pins it after the real exp in the ScalarE queue
            nc.scalar.activation(out=dm2, in_=exp_sb[0:1, 0:1], func=AF.Sqrt)

            # ---------- phase 3: weighted sum  out[c,p] = sum_d a_d * FF_d ---
            with tc.tile_pool(name="ps_acc", bufs=1, space="PSUM") as pa:
                for ct in range(2):
                    acc = pa.tile([128, P], fp32, tag=f"acc{ct}")
                    for d, (di, dj) in enumerate(SHIFTS):
                        veng = nc.gpsimd if d == 8 else nc.vector
                        prod = wp.tile([128, P], bf16, tag="prod")
                        pv = prod.rearrange("c (r w) -> c r w", w=W)
                        veng.tensor_tensor(
                            out=pv,
                            in0=ffview(ct, di, dj),
                            in1=rep_sb[d].rearrange("c (r w) -> c r w", w=W),
                            op=Alu.mult,
                        )
                        for (o, n) in CHUNKS:
                            nc.tensor.matmul(
                                acc[:, o : o + n], ident, prod[:, o : o + n],
                                start=(d == 0), stop=(d == 8),
                            )
                    # deferred softmax denominator: oat = acc * (1/den)
                    oat = sp.tile([128, P], bf16, tag=f"oat{ct}", name=f"oat{ct}")
                    nc.vector.tensor_tensor(out=oat, in0=acc, in1=rec_rep, op=Alu.mult)
                    eng = dma if ct == 0 else nc.scalar
                    eng.dma_start(out=a2a_in[4 * ct : 4 * ct + 4, :, :], in_=oat)

            # ---------- AllToAll ----------
            nc.gpsimd.collective_compute(
                "AllToAll", Alu.bypass, replica_groups=GROUPS,
                ins=[a2a_in[:, :, :]], outs=[a2a_out[:, :, :]],
            )
            # prefetch the first down-matmul weights into the PE while the
            # a2a is in flight, so the first post-a2a matmul skips LDWEIGHTS
            nc.tensor.ldweights(dw_sb[0][:, 0:128])

            # ---------- down matmul: strided rhs de-scramble on read --------
            # rhs[kt][(j,b), s, t] = a2a_out[4kt+j, s, 36b+t]
            rhs_sb = [sp.tile([128, 32, 36], bf16, tag=f"rhs{kt}", name=f"rhs{kt}") for kt in range(2)]
            rhs_engs = [dma, nc.scalar, nc.gpsimd]
            a2a_bv = a2a_out.rearrange("j s (b t) -> j b s t", t=36)
            for kt in range(2):
                for jj in range(4):
                    rhs_engs[(4 * kt + jj) % 3].dma_start(
                        out=rhs_sb[kt][32 * jj : 32 * jj + 32, :, :],
                        in_=a2a_bv[4 * kt + jj],
                    )

            stat_sb = sp.tile([128, 4], fp32, tag="stat")
            sq_scr = wp.tile([128, P], fp32, tag="sqscr")
            zo_sb = [sp.tile([128, P], bf16, tag=f"zo{mt}", name=f"zo{mt}") for mt in range(2)]
            with tc.tile_pool(name="ps_z", bufs=1, space="PSUM") as pz:
                z_ps = [pz.tile([128, P], fp32, tag=f"z{mt}", name=f"z{mt}") for mt in range(2)]
                # kt-outer: all kt0 passes run while the strided kt1 rhs
                # loads are still landing; weights reload only 4x total
                for kt in range(2):
                    for mt in range(2):
                        for (o, n) in CHUNKS:
                            nc.tensor.matmul(
                                z_ps[mt][:, o : o + n],
                                dw_sb[kt][:, 128 * mt : 128 * mt + 128],
                                rhs_sb[kt].rearrange("c s t -> c (s t)")[:, o : o + n],
                                start=(kt == 0), stop=(kt == 1),
                            )
                for mt in range(2):
                    nc.vector.tensor_reduce(
                        out=stat_sb[:, mt : mt + 1], in_=z_ps[mt],
                        axis=mybir.AxisListType.X, op=Alu.add,
                    )
                    nc.scalar.activation(
                        out=sq_scr, in_=z_ps[mt], func=AF.Square,
                        accum_out=stat_sb[:, 2 + mt : 3 + mt],
                    )
                nc.gpsimd.dma_start(out=st_part[:, :], in_=stat_sb)
                nc.gpsimd.collective_compute(
                    "AllReduce", Alu.add, replica_groups=GROUPS,
                    ins=[st_part[:, :]], outs=[st_sum[:, :]],
                )
                gl_sb = sp.tile([128, 4], fp32, tag="glstat")
                nc.gpsimd.dma_start(out=gl_sb, in_=st_sum[:, :])

                # mu = sum/HW ; var = sumsq/HW - mu^2 ; inv = rsqrt(var+eps)
                ins_sb = sp.tile([128, 8], fp32, tag="instat")
                g4 = ins_sb[:, 0:4]         # [mu0, mu1, e20, e21]
                mu2 = ins_sb[:, 0:2]
                e22 = ins_sb[:, 2:4]
                inv2 = ins_sb[:, 4:6]
                nmi2 = ins_sb[:, 6:8]
                nc.vector.tensor_scalar(out=g4, in0=gl_sb[:, 0:4],
                                        scalar1=1.0 / HW, scalar2=None, op0=Alu.mult)
                nc.vector.tensor_tensor(out=inv2, in0=mu2, in1=mu2, op=Alu.mult)
                nc.vector.tensor_tensor(out=e22, in0=e22, in1=inv2, op=Alu.subtract)
                nc.scalar.activation(out=e22, in_=e22, func=AF.Sqrt, bias=eps_sb, scale=1.0)
                nc.vector.reciprocal(out=inv2, in_=e22)
                nc.vector.scalar_tensor_tensor(out=nmi2, in0=mu2, scalar=-1.0,
                                               in1=inv2, op0=Alu.mult, op1=Alu.mult)
                # LeakyReLU((z - mu) * inv): mt0 fused on ScalarE as
                # prelu(z*inv + (-mu*inv), alpha=0.2); mt1 on DVE as
                # w = z*inv + nmi ; out = max(w, 0.2*w) — the two halves run
                # on different engines concurrently.
                nc.scalar.activation(
                    out=zo_sb[0], in_=z_ps[0], func=AF.Prelu,
                    bias=ins_sb[:, 6:7], scale=ins_sb[:, 4:5], alpha=0.2,
                )
                dma.dma_start(
                    out=outp[0:128, :, :],
                    in_=zo_sb[0].rearrange("c (s t) -> c s t", t=36),
                )
                w_sb = wp.tile([128, P], fp32, tag="wnorm")
                nc.vector.tensor_scalar(
                    out=w_sb, in0=z_ps[1], scalar1=ins_sb[:, 5:6],
                    scalar2=ins_sb[:, 7:8], op0=Alu.mult, op1=Alu.add,
                )
                nc.vector.scalar_tensor_tensor(
                    out=zo_sb[1], in0=w_sb, scalar=0.2, in1=w_sb,
                    op0=Alu.mult, op1=Alu.max,
                )
                nc.scalar.dma_start(
                    out=outp[128:256, :, :],
                    in_=zo_sb[1].rearrange("c (s t) -> c s t", t=36),
                )
    nc.compile()
    return nc


def _get_nc():
    if "nc" not in _cache:
        _cache["nc"] = _build()
    return _cache["nc"]


def _shard_inputs(x, se_w1, se_b1, se_w2, se_b2, down_w):
    import ml_dtypes

    x = np.ascontiguousarray(np.asarray(x, np.float32))[0]          # (C, H, W)
    mean = x.mean(axis=(1, 2))                                      # (C,)
    mn = np.ascontiguousarray(mean.reshape(2, 128).T).astype(np.float32)
    w1t = np.ascontiguousarray(np.asarray(se_w1, np.float32).T)     # (C, 16)
    b1 = np.ascontiguousarray(np.asarray(se_b1, np.float32)[:, None])
    w2t = np.ascontiguousarray(np.asarray(se_w2, np.float32).T)     # (16, C)
    b2 = np.ascontiguousarray(np.asarray(se_b2, np.float32)[:, None])
    dwt = np.ascontiguousarray(
        np.asarray(down_w, np.float32).T.astype(ml_dtypes.bfloat16)
    )                                                               # (C, C) bf16

    in_maps = []
    for k in range(M):
        slab = np.zeros((C, SLAB, W), ml_dtypes.bfloat16)
        lo, hi = RPC * k - 1, RPC * k + RPC + 1
        clo, chi = max(lo, 0), min(hi, H)
        slab[:, clo - lo : clo - lo + (chi - clo), :] = x[:, clo:chi, :].astype(
            ml_dtypes.bfloat16
        )
        msk = np.ones((128, 2), np.float32)
        if k == 0:
            msk[:, 0] = 0.0
        if k == M - 1:
            msk[:, 1] = 0.0
        in_maps.append({
            "xs": slab, "mn": mn, "msk": msk, "w1t": w1t, "b1": b1,
            "w2t": w2t, "b2": b2, "dwt": dwt,
        })
    return in_maps


def _gather(results):
    R = np.stack([np.asarray(r["out"]).astype(np.float32) for r in results])
    return np.ascontiguousarray(
        R.transpose(1, 3, 0, 2).reshape(1, C, H, W).astype(np.float32)
    )


def kernel(x, se_w1, se_b1, se_w2, se_b2, down_w, _trace=False):
    from concourse.bass_utils import run_bass_kernel_spmd

    nc = _get_nc()
    in_maps = _shard_inputs(x, se_w1, se_b1, se_w2, se_b2, down_w)
    res = run_bass_kernel_spmd(nc, in_maps, core_ids=list(range(M)), trace=_trace)
    out = _gather(res.results)
    if _trace:
        kernel.last_results = res
    return out


# revision 37
# speedup vs baseline: 4.5717x; 4.5717x over previous
"""Trainium2 Bass kernel for the SE + patch-correlation-attention + down-conv module.

Sharding (8 cores): split the 96 image rows into 8 slabs of 12 rows.

Execution-time model (measured): the FIRST collective of every NEFF execution
completes ~90us after exec start regardless of issue time; later collectives
take ~10us. So a dummy warm-up AllReduce is issued at t=0 and the entire
attention pipeline (which has no cross-core dependency once the SE gate is
known) runs hidden under that window. The SE gate is computed on-device from a
host-precomputed global channel mean (the only cross-core quantity), so no
collective is needed before the AllToAll.

Per core:
  1. warm-up AllReduce (absorbs the ~90us collective-path init)
  2. SE gate y from host-shipped channel mean (tiny MLP on device)
  3. FF = x*y, S = sigmoid(x*y) on the 14-row halo slab (bf16)
  4. patch correlation (9 shifts, bf16 DVE products + PE one-hot reduction),
     softmax (pre-normalized rows), weighted sum -> attention out (256 x 1152)
  5. the a2a staging write scrambles to (dest, b, s, t) layout on the SENDER
     (descriptor-heavy, but hidden pre-a2a) so the post-a2a rhs loads are
     contiguous
  6. AllToAll (bf16), then contiguous rhs loads, 256x256 down matmul,
     InstanceNorm partials -> tiny AllReduce, normalize + LeakyReLU,
     write the (256, 32, 36) bf16 output slice
Host: computes the channel mean, shards x (bf16), gathers + permutes output.

ScalarE activation-table loads (~2.7us each) are prefetched with dummy 1-elem
activations so no table switch lands on the critical path.
"""
import numpy as np

C, H, W, M = 256, 96, 96, 8
RPC = H // M          # 12 rows per core
P = RPC * W           # 1152 positions per core
SLAB = RPC + 2        # 14 rows incl. halo
WP = 100              # padded slab width (j0 at col 2)
HW = H * W            # 9216
SHIFTS = [(di, dj) for di in (-1, 0, 1) for dj in (-1, 0, 1)]
CHUNKS = [(0, 512), (512, 512), (1024, 128)]   # psum-bank-aligned matmul N-chunks

_cache = {}


def _build():
    import concourse.bass as bass
    from concourse import bacc
    import concourse.mybir as mybir
    from concourse.tile import TileContext
    from concourse.masks import make_identity

    fp32 = mybir.dt.float32
    bf16 = mybir.dt.bfloat16
    AF = mybir.ActivationFunctionType
    Alu = mybir.AluOpType
    GROUPS = [list(range(M))]

    nc = bacc.Bacc()

    xs = nc.declare_dram_parameter("xs", [C, SLAB, W], bf16, isOutput=False)
    mn = nc.declare_dram_parameter("mn", [128, 2], fp32, isOutput=False)
    msk = nc.declare_dram_parameter("msk", [128, 2], fp32, isOutput=False)
    w1t = nc.declare_dram_parameter("w1t", [C, 16], fp32, isOutput=False)
    b1 = nc.declare_dram_parameter("b1", [16, 1], fp32, isOutput=False)
    w2t = nc.declare_dram_parameter("w2t", [16, C], fp32, isOutput=False)
    b2 = nc.declare_dram_parameter("b2", [C, 1], fp32, isOutput=False)
    dwt = nc.declare_dram_parameter("dwt", [C, C], bf16, isOutput=False)
    outp = nc.declare_dram_parameter("out", [C, 32, 36], bf16, isOutput=True)

    warm_in = nc.dram_tensor("warm_in", [1, 1], fp32)
    warm_out = nc.dram_tensor("warm_out", [1, 1], fp32, addr_space="Shared")
    # a2a chunks in (dest, s, p) layout: s = attention channel within the
    # dest's 32-slice, p = sender-local position; the receiver de-scrambles
    # with strided DRAM reads (cheaper than write-side scrambling)
    a2a_in = nc.dram_tensor("a2a_in", [M, 32, P], bf16)
    a2a_out = nc.dram_tensor("a2a_out", [M, 32, P], bf16)
    st_part = nc.dram_tensor("st_part", [128, 4], fp32)
    st_sum = nc.dram_tensor("st_sum", [128, 4], fp32, addr_space="Shared")
    a_dram = nc.dram_tensor("a_dram", [9, P], bf16)
    r_dram = nc.dram_tensor("r_dram", [P], fp32)
    dma = nc.default_dma_engine

    with TileContext(nc) as tc:
        with (
            tc.tile_pool(name="const", bufs=1) as cp,
            tc.tile_pool(name="sb", bufs=1) as sp,
            tc.tile_pool(name="work", bufs=6) as wp,
        ):
            # ---------- warm up the collective path before anything else ----
            # (the first collective of each execution pays a large, variable
            # init cost; a dummy AllReduce at t=0 absorbs it concurrently
            # with the compute below — removing it was measured to produce
            # catastrophic outliers on the real AllToAll)
            nc.gpsimd.collective_compute(
                "AllReduce", Alu.add, replica_groups=GROUPS,
                ins=[warm_in[:, :]], outs=[warm_out[:, :]],
            )

            # ---------- small consts first (y-gate path), then x slabs ------
            mn_sb = cp.tile([128, 2], fp32, tag="mn")
            dma.dma_start(out=mn_sb, in_=mn[:, :])
            b1_sb = cp.tile([16, 1], fp32)
            nc.scalar.dma_start(out=b1_sb, in_=b1[:, :])
            b2_sb = cp.tile([128, 2], fp32)
            w1_sb = [cp.tile([128, 16], fp32, tag=f"w1_{ct}", name=f"w1_{ct}") for ct in range(2)]
            for ct in range(2):
                nc.scalar.dma_start(out=b2_sb[:, ct : ct + 1], in_=b2[128 * ct : 128 * ct + 128, :])
                dma.dma_start(out=w1_sb[ct], in_=w1t[128 * ct : 128 * ct + 128, :])
            w2_sb = cp.tile([16, C], fp32)
            dma.dma_start(out=w2_sb, in_=w2t[:, :])

            x_sb = [sp.tile([128, SLAB, W], bf16, tag=f"x{ct}", name=f"x{ct}") for ct in range(2)]
            dma.dma_start(out=x_sb[0], in_=xs[0:128, :, :])
            nc.scalar.dma_start(out=x_sb[1], in_=xs[128:256, :, :])

            msk_sb = cp.tile([128, 2], fp32)
            nc.gpsimd.dma_start(out=msk_sb, in_=msk[:, :])
            dw_sb = [cp.tile([128, C], bf16, tag=f"dw_{ct}", name=f"dw_{ct}") for ct in range(2)]
            for ct in range(2):
                nc.gpsimd.dma_start(out=dw_sb[ct], in_=dwt[128 * ct : 128 * ct + 128, :])

            # ---------- constants ----------
            ident = cp.tile([128, 128], bf16)
            make_identity(nc, ident)
            e_all = cp.tile([128, 9, 9], bf16)
            nc.gpsimd.memset(e_all, 0.0)
            for d in range(9):
                nc.gpsimd.memset(e_all[:, d, d : d + 1], 1.0)
            ones_99 = cp.tile([9, 9], bf16)
            nc.gpsimd.memset(ones_99, 1.0)
            eps_sb = cp.tile([128, 1], fp32)
            nc.gpsimd.memset(eps_sb, 1e-5)
            dm1 = cp.tile([1, 1], fp32)
            dm2 = cp.tile([1, 1], fp32)
            nc.vector.memset(dm1, 0.0)
            nc.vector.memset(dm2, 0.0)

            # ---------- SE gate from host-shipped channel mean ----------
            with tc.tile_pool(name="ps_se", bufs=1, space="PSUM") as pse:
                h_ps = pse.tile([16, 1], fp32)
                for ct in range(2):
                    nc.tensor.matmul(
                        h_ps, w1_sb[ct], mn_sb[:, ct : ct + 1],
                        start=(ct == 0), stop=(ct == 1),
                    )
                h_sb = sp.tile([16, 1], fp32)
                # relu on DVE (avoids burning a ScalarE table slot on Relu)
                nc.vector.tensor_scalar(
                    out=h_sb, in0=h_ps, scalar1=b1_sb[:, 0:1], scalar2=0.0,
                    op0=Alu.add, op1=Alu.max,
                )
                y_ps = pse.tile([128, 2], fp32)
                y_sb = sp.tile([128, 2], fp32, tag="ygate")
                for ct in range(2):
                    nc.tensor.matmul(
                        y_ps[:, ct : ct + 1], w2_sb[:, 128 * ct : 128 * ct + 128], h_sb,
                        start=True, stop=True,
                    )
                    # first ScalarE op in the program: loads the sigmoid table
                    # set while the x slabs are still streaming in
                    nc.scalar.activation(out=y_sb[:, ct : ct + 1], in_=y_ps[:, ct : ct + 1],
                                         func=AF.Sigmoid, bias=b2_sb[:, ct : ct + 1], scale=1.0)

            # ---------- FF and S maps (bf16, zero-padded 14x100 layout) ------
            ff_sb = [sp.tile([128, SLAB, WP], bf16, tag=f"ff{ct}", name=f"ff{ct}") for ct in range(2)]
            s_sb = [sp.tile([128, SLAB, WP], bf16, tag=f"s{ct}", name=f"s{ct}") for ct in range(2)]
            ff2_sb = [sp.tile([128, SLAB, WP], bf16, tag=f"ff2{ct}", name=f"ff2{ct}") for ct in range(2)]
            s2_sb = [sp.tile([128, SLAB, WP], bf16, tag=f"s2{ct}", name=f"s2{ct}") for ct in range(2)]
            for ct in range(2):
                nc.vector.memset(s2_sb[ct][:, :, 2:3], 0.0)
                nc.vector.memset(s2_sb[ct][:, :, 99:100], 0.0)
                nc.vector.memset(ff2_sb[ct][:, :, 2:3], 0.0)
                nc.vector.memset(ff2_sb[ct][:, :, 99:100], 0.0)
                nc.vector.tensor_scalar(
                    out=ff_sb[ct][:, :, 2:98], in0=x_sb[ct],
                    scalar1=y_sb[:, ct : ct + 1], scalar2=None, op0=Alu.mult,
                )
                nc.scalar.activation(
                    out=s_sb[ct][:, :, 2:98], in_=x_sb[ct],
                    func=AF.Sigmoid, scale=y_sb[:, ct : ct + 1],
                )
                nc.vector.tensor_scalar(
                    out=s_sb[ct][:, 0, 2:98], in0=s_sb[ct][:, 0, 2:98],
                    scalar1=msk_sb[:, 0:1], scalar2=None, op0=Alu.mult,
                )
                nc.vector.tensor_scalar(
                    out=s_sb[ct][:, 13, 2:98], in0=s_sb[ct][:, 13, 2:98],
                    scalar1=msk_sb[:, 1:2], scalar2=None, op0=Alu.mult,
                )
                nc.vector.tensor_copy(out=s2_sb[ct][:, :, 3:99], in_=s_sb[ct][:, :, 2:98])
                nc.vector.tensor_copy(out=ff2_sb[ct][:, :, 3:99], in_=ff_sb[ct][:, :, 2:98])

            # preload the exp table set while phase 2 runs; the input is a
            # slice of the ct1 S map purely to pin this op's position in the
            # ScalarE queue (after the S sigmoids, before the real exp)
            nc.scalar.activation(out=dm1, in_=s_sb[1][0:1, 0, 2:3], func=AF.Exp)

            def sview(ct, di, dj):
                if dj == 0:
                    return s_sb[ct][:, 1 + di : 13 + di, 2:98]
                return s2_sb[ct][:, 1 + di : 13 + di, 3 + dj : 99 + dj]

            def ffview(ct, di, dj):
                if dj == 0:
                    return ff_sb[ct][:, 1 + di : 13 + di, 2:98]
                return ff2_sb[ct][:, 1 + di : 13 + di, 3 + dj : 99 + dj]

            # ---------- phase 2: correlation  A[d, p] = sum_c S*S_d ----------
            with tc.tile_pool(name="ps_corr", bufs=1, space="PSUM") as pc:
                A_ps = pc.tile([9, P], fp32)
                for d, (di, dj) in enumerate(SHIFTS):
                    for ct in range(2):
                        # all products on DVE: GpSimd is ~3x slower per op and
                        # pays a ~4us program-load penalty on its first use,
                        # which made it the phase-2 tail when offloaded here
                        veng = nc.vector
                        prod = wp.tile([128, P], bf16, tag="prod")
                        pv = prod.rearrange("c (r w) -> c r w", w=W)
                        veng.tensor_tensor(
                            out=pv,
                            in0=s_sb[ct][:, 1:13, 2:98],
                            in1=sview(ct, di, dj),
                            op=Alu.mult,
                        )
                        for (o, n) in CHUNKS:
                            nc.tensor.matmul(
                                A_ps[:, o : o + n], e_all[:, d, :], prod[:, o : o + n],
                                start=(d == 0 and ct == 0), stop=(d == 8 and ct == 1),
                            )
                # softmax numerator: exp with the folded 1/C mean.  The
                # 1/denominator is deferred to after the phase-3 psum
                # accumulation so the row broadcasts start immediately.
                exp_sb = sp.tile([9, P], bf16, tag="exps")
                nc.scalar.activation(out=exp_sb, in_=A_ps, func=AF.Exp, scale=1.0 / C)
                dma.dma_start(out=a_dram[:, :], in_=exp_sb)
                den_ps = pc.tile([9, P], fp32)
                for (o, n) in CHUNKS:
                    nc.tensor.matmul(den_ps[:, o : o + n], ones_99, exp_sb[:, o : o + n],
                                     start=True, stop=True)
                rec_sb = sp.tile([9, P], fp32, tag="recs")
                nc.vector.reciprocal_approx_fast(out=rec_sb, in_=den_ps)

            # ---------- replicate exp rows + recip across 128 partitions ----
            rep_sb = [sp.tile([128, P], bf16, tag=f"rep{d}", name=f"rep{d}") for d in range(9)]
            for d in range(9):
                eng = dma if d % 2 == 0 else nc.scalar
                eng.dma_start(
                    out=rep_sb[d],
                    in_=a_dram[d, :].partition_broadcast(128),
                )
            rec_row = sp.tile([1, P], fp32, tag="recrow")
            nc.vector.tensor_copy(out=rec_row, in_=rec_sb[0:1, :])
            nc.scalar.dma_start(out=r_dram[:], in_=rec_row)
            rec_rep = sp.tile([128, P], fp32, tag="recrep")
            nc.scalar.dma_start(out=rec_rep, in_=r_dram[:].partition_broadcast(128))
            # preload the sqrt/prelu table set during phase 3 / a2a; input
            # slice of exp_sb pins it after the real exp in the ScalarE queue
            nc.scalar.activation(out=dm2, in_=exp_sb[0:1, 0:1], func=AF.Sqrt)

            # ---------- phase 3: weighted sum  out[c,p] = sum_d a_d * FF_d ---
            with tc.tile_pool(name="ps_acc", bufs=1, space="PSUM") as pa:
                for ct in range(2):
                    acc = pa.tile([128, P], fp32, tag=f"acc{ct}")
                    for d, (di, dj) in enumerate(SHIFTS):
                        veng = nc.gpsimd if d == 8 else nc.vector
                        prod = wp.tile([128, P], bf16, tag="prod")
                        pv = prod.rearrange("c (r w) -> c r w", w=W)
                        veng.tensor_tensor(
                            out=pv,
                            in0=ffview(ct, di, dj),
                            in1=rep_sb[d].rearrange("c (r w) -> c r w", w=W),
                            op=Alu.mult,
                        )
                        for (o, n) in CHUNKS:
                            nc.tensor.matmul(
                                acc[:, o : o + n], ident, prod[:, o : o + n],
                                start=(d == 0), stop=(d == 8),
                            )
                    # deferred softmax denominator: oat = acc * (1/den)
                    oat = sp.tile([128, P], bf16, tag=f"oat{ct}", name=f"oat{ct}")
                    nc.vector.tensor_tensor(out=oat, in0=acc, in1=rec_rep, op=Alu.mult)
                    eng = dma if ct == 0 else nc.scalar
                    eng.dma_start(out=a2a_in[4 * ct : 4 * ct + 4, :, :], in_=oat)

            # ---------- AllToAll ----------
            nc.gpsimd.collective_compute(
                "AllToAll", Alu.bypass, replica_groups=GROUPS,
                ins=[a2a_in[:, :, :]], outs=[a2a_out[:, :, :]],
            )
            # prefetch the first down-matmul weights into the PE while the
            # a2a is in flight, so the first post-a2a matmul skips LDWEIGHTS
            nc.tensor.ldweights(dw_sb[0][:, 0:128])

            # ---------- down matmul: strided rhs de-scramble on read --------
            # rhs[kt][(j,b), s, t] = a2a_out[4kt+j, s, 36b+t]
            rhs_sb = [sp.tile([128, 32, 36], bf16, tag=f"rhs{kt}", name=f"rhs{kt}") for kt in range(2)]
            rhs_engs = [dma, nc.scalar, nc.gpsimd]
            a2a_bv = a2a_out.rearrange("j s (b t) -> j b s t", t=36)
            for kt in range(2):
                for jj in range(4):
                    rhs_engs[(4 * kt + jj) % 3].dma_start(
                        out=rhs_sb[kt][32 * jj : 32 * jj + 32, :, :],
                        in_=a2a_bv[4 * kt + jj],
                    )

            stat_sb = sp.tile([128, 4], fp32, tag="stat")
            sq_scr = wp.tile([128, P], fp32, tag="sqscr")
            zo_sb = [sp.tile([128, P], bf16, tag=f"zo{mt}", name=f"zo{mt}") for mt in range(2)]
            with tc.tile_pool(name="ps_z", bufs=1, space="PSUM") as pz:
                z_ps = [pz.tile([128, P], fp32, tag=f"z{mt}", name=f"z{mt}") for mt in range(2)]
                # kt-outer: all kt0 passes run while the strided kt1 rhs
                # loads are still landing; weights reload only 4x total
                for kt in range(2):
                    for mt in range(2):
                        for (o, n) in CHUNKS:
                            nc.tensor.matmul(
                                z_ps[mt][:, o : o + n],
                                dw_sb[kt][:, 128 * mt : 128 * mt + 128],
                                rhs_sb[kt].rearrange("c s t -> c (s t)")[:, o : o + n],
                                start=(kt == 0), stop=(kt == 1),
                            )
                for mt in range(2):
                    nc.vector.tensor_reduce(
                        out=stat_sb[:, mt : mt + 1], in_=z_ps[mt],
                        axis=mybir.AxisListType.X, op=Alu.add,
                    )
                    nc.scalar.activation(
                        out=sq_scr, in_=z_ps[mt], func=AF.Square,
                        accum_out=stat_sb[:, 2 + mt : 3 + mt],
                    )
                nc.gpsimd.dma_start(out=st_part[:, :], in_=stat_sb)
                nc.gpsimd.collective_compute(
                    "AllReduce", Alu.add, replica_groups=GROUPS,
                    ins=[st_part[:, :]], outs=[st_sum[:, :]],
                )
                gl_sb = sp.tile([128, 4], fp32, tag="glstat")
                nc.gpsimd.dma_start(out=gl_sb, in_=st_sum[:, :])

                # mu = sum/HW ; var = sumsq/HW - mu^2 ; inv = rsqrt(var+eps)
                ins_sb = sp.tile([128, 8], fp32, tag="instat")
                g4 = ins_sb[:, 0:4]         # [mu0, mu1, e20, e21]
                mu2 = ins_sb[:, 0:2]
                e22 = ins_sb[:, 2:4]
                inv2 = ins_sb[:, 4:6]
                nmi2 = ins_sb[:, 6:8]
                nc.vector.tensor_scalar(out=g4, in0=gl_sb[:, 0:4],
                                        scalar1=1.0 / HW, scalar2=None, op0=Alu.mult)
                nc.vector.tensor_tensor(out=inv2, in0=mu2, in1=mu2, op=Alu.mult)
                nc.vector.tensor_tensor(out=e22, in0=e22, in1=inv2, op=Alu.subtract)
                nc.scalar.activation(out=e22, in_=e22, func=AF.Sqrt, bias=eps_sb, scale=1.0)
                nc.vector.reciprocal(out=inv2, in_=e22)
                nc.vector.scalar_tensor_tensor(out=nmi2, in0=mu2, scalar=-1.0,
                                               in1=inv2, op0=Alu.mult, op1=Alu.mult)
                # LeakyReLU((z - mu) * inv): mt0 fused on ScalarE as
                # prelu(z*inv + (-mu*inv), alpha=0.2); mt1 on DVE as
                # w = z*inv + nmi ; out = max(w, 0.2*w) — the two halves run
                # on different engines concurrently.
                nc.scalar.activation(
                    out=zo_sb[0], in_=z_ps[0], func=AF.Prelu,
                    bias=ins_sb[:, 6:7], scale=ins_sb[:, 4:5], alpha=0.2,
                )
                dma.dma_start(
                    out=outp[0:128, :, :],
                    in_=zo_sb[0].rearrange("c (s t) -> c s t", t=36),
                )
                w_sb = wp.tile([128, P], fp32, tag="wnorm")
                nc.vector.tensor_scalar(
                    out=w_sb, in0=z_ps[1], scalar1=ins_sb[:, 5:6],
                    scalar2=ins_sb[:, 7:8], op0=Alu.mult, op1=Alu.add,
                )
                nc.vector.scalar_tensor_tensor(
                    out=zo_sb[1], in0=w_sb, scalar=0.2, in1=w_sb,
                    op0=Alu.mult, op1=Alu.max,
                )
                nc.scalar.dma_start(
                    out=outp[128:256, :, :],
                    in_=zo_sb[1].rearrange("c (s t) -> c s t", t=36),
                )
    nc.compile()
    return nc


def _get_nc():
    if "nc" not in _cache:
        _cache["nc"] = _build()
    return _cache["nc"]


def _shard_inputs(x, se_w1, se_b1, se_w2, se_b2, down_w):
    import ml_dtypes

    x = np.ascontiguousarray(np.asarray(x, np.float32))[0]          # (C, H, W)
    mean = x.mean(axis=(1, 2))                                      # (C,)
    mn = np.ascontiguousarray(mean.reshape(2, 128).T).astype(np.float32)
    w1t = np.ascontiguousarray(np.asarray(se_w1, np.float32).T)     # (C, 16)
    b1 = np.ascontiguousarray(np.asarray(se_b1, np.float32)[:, None])
    w2t = np.ascontiguousarray(np.asarray(se_w2, np.float32).T)     # (16, C)
    b2 = np.ascontiguousarray(np.asarray(se_b2, np.float32)[:, None])
    dwt = np.ascontiguousarray(
        np.asarray(down_w, np.float32).T.astype(ml_dtypes.bfloat16)
    )                                                               # (C, C) bf16

    in_maps = []
    for k in range(M):
        slab = np.zeros((C, SLAB, W), ml_dtypes.bfloat16)
        lo, hi = RPC * k - 1, RPC * k + RPC + 1
        clo, chi = max(lo, 0), min(hi, H)
        slab[:, clo - lo : clo - lo + (chi - clo), :] = x[:, clo:chi, :].astype(
            ml_dtypes.bfloat16
        )
        msk = np.ones((128, 2), np.float32)
        if k == 0:
            msk[:, 0] = 0.0
        if k == M - 1:
            msk[:, 1] = 0.0
        in_maps.append({
            "xs": slab, "mn": mn, "msk": msk, "w1t": w1t, "b1": b1,
            "w2t": w2t, "b2": b2, "dwt": dwt,
        })
    return in_maps


def _gather(results):
    R = np.stack([np.asarray(r["out"]).astype(np.float32) for r in results])
    return np.ascontiguousarray(
        R.transpose(1, 3, 0, 2).reshape(1, C, H, W).astype(np.float32)
    )


def kernel(x, se_w1, se_b1, se_w2, se_b2, down_w, _trace=False):
    from concourse.bass_utils import run_bass_kernel_spmd

    nc = _get_nc()
    in_maps = _shard_inputs(x, se_w1, se_b1, se_w2, se_b2, down_w)
    res = run_bass_kernel_spmd(nc, in_maps, core_ids=list(range(M)), trace=_trace)
    out = _gather(res.results)
    if _trace:
        kernel.last_results = res
    return out


# revision 41
# speedup vs baseline: 4.8975x; 1.0712x over previous
"""Trainium2 Bass kernel for the SE + patch-correlation-attention + down-conv module.

Sharding (8 cores): split the 96 image rows into 8 slabs of 12 rows.

Execution-time model (measured): the FIRST collective of every NEFF execution
completes ~90us after exec start regardless of issue time; later collectives
take ~10us. So a dummy warm-up AllReduce is issued at t=0 and the entire
attention pipeline (which has no cross-core dependency once the SE gate is
known) runs hidden under that window. The SE gate is computed on-device from a
host-precomputed global channel mean (the only cross-core quantity), so no
collective is needed before the AllToAll.

Per core:
  1. warm-up AllReduce (absorbs the ~90us collective-path init)
  2. SE gate y from host-shipped channel mean (tiny MLP on device)
  3. FF = x*y, S = sigmoid(x*y) on the 14-row halo slab (bf16)
  4. patch correlation (9 shifts, bf16 DVE products + PE one-hot reduction),
     softmax (pre-normalized rows), weighted sum -> attention out (256 x 1152)
  5. the a2a staging write scrambles to (dest, b, s, t) layout on the SENDER
     (descriptor-heavy, but hidden pre-a2a) so the post-a2a rhs loads are
     contiguous
  6. AllToAll (bf16), then contiguous rhs loads, 256x256 down matmul,
     InstanceNorm partials -> tiny AllReduce, normalize + LeakyReLU,
     write the (256, 32, 36) bf16 output slice
Host: computes the channel mean, shards x (bf16), gathers + permutes output.

ScalarE activation-table loads (~2.7us each) are prefetched with dummy 1-elem
activations so no table switch lands on the critical path.
"""
import numpy as np

C, H, W, M = 256, 96, 96, 8
RPC = H // M          # 12 rows per core
P = RPC * W           # 1152 positions per core
SLAB = RPC + 2        # 14 rows incl. halo
WP = 100              # padded slab width (j0 at col 2)
HW = H * W            # 9216
SHIFTS = [(di, dj) for di in (-1, 0, 1) for dj in (-1, 0, 1)]
CHUNKS = [(0, 512), (512, 512), (1024, 128)]   # psum-bank-aligned matmul N-chunks

_cache = {}


def _build():
    import concourse.bass as bass
    from concourse import bacc
    import concourse.mybir as mybir
    from concourse.tile import TileContext
    from concourse.masks import make_identity

    fp32 = mybir.dt.float32
    bf16 = mybir.dt.bfloat16
    AF = mybir.ActivationFunctionType
    Alu = mybir.AluOpType
    GROUPS = [list(range(M))]

    nc = bacc.Bacc()

    xs = nc.declare_dram_parameter("xs", [C, SLAB, W], bf16, isOutput=False)
    mn = nc.declare_dram_parameter("mn", [128, 2], fp32, isOutput=False)
    msk = nc.declare_dram_parameter("msk", [128, 2], fp32, isOutput=False)
    w1t = nc.declare_dram_parameter("w1t", [C, 16], fp32, isOutput=False)
    b1 = nc.declare_dram_parameter("b1", [16, 1], fp32, isOutput=False)
    w2t = nc.declare_dram_parameter("w2t", [16, C], fp32, isOutput=False)
    b2 = nc.declare_dram_parameter("b2", [C, 1], fp32, isOutput=False)
    dwt = nc.declare_dram_parameter("dwt", [C, C], bf16, isOutput=False)
    outp = nc.declare_dram_parameter("out", [C, 32, 36], bf16, isOutput=True)

    warm_in = nc.dram_tensor("warm_in", [1, 1], fp32)
    warm_out = nc.dram_tensor("warm_out", [1, 1], fp32, addr_space="Shared")
    # a2a chunks in (dest, s, p) layout: s = attention channel within the
    # dest's 32-slice, p = sender-local position; the receiver de-scrambles
    # with strided DRAM reads (cheaper than write-side scrambling; a strided
    # collective-out AP that would scatter on receive fails neuronx compile)
    a2a_in = nc.dram_tensor("a2a_in", [M, 32, P], bf16)
    a2a_out = nc.dram_tensor("a2a_out", [M, 32, P], bf16)
    st_part = nc.dram_tensor("st_part", [128, 4], fp32)
    st_sum = nc.dram_tensor("st_sum", [128, 4], fp32, addr_space="Shared")
    a_dram = nc.dram_tensor("a_dram", [9, P], bf16)
    r_dram = nc.dram_tensor("r_dram", [P], fp32)
    dma = nc.default_dma_engine

    with TileContext(nc) as tc:
        with (
            tc.tile_pool(name="const", bufs=1) as cp,
            tc.tile_pool(name="sb", bufs=1) as sp,
            tc.tile_pool(name="work", bufs=6) as wp,
        ):
            # ---------- warm up the collective path before anything else ----
            # (the first collective of each execution pays a large, variable
            # init cost; a dummy AllReduce at t=0 absorbs it concurrently
            # with the compute below — removing it was measured to produce
            # catastrophic outliers on the real AllToAll)
            nc.gpsimd.collective_compute(
                "AllReduce", Alu.add, replica_groups=GROUPS,
                ins=[warm_in[:, :]], outs=[warm_out[:, :]],
            )

            # ---------- small consts first (y-gate path), then x slabs ------
            mn_sb = cp.tile([128, 2], fp32, tag="mn")
            dma.dma_start(out=mn_sb, in_=mn[:, :])
            b1_sb = cp.tile([16, 1], fp32)
            nc.scalar.dma_start(out=b1_sb, in_=b1[:, :])
            b2_sb = cp.tile([128, 2], fp32)
            w1_sb = [cp.tile([128, 16], fp32, tag=f"w1_{ct}", name=f"w1_{ct}") for ct in range(2)]
            for ct in range(2):
                nc.scalar.dma_start(out=b2_sb[:, ct : ct + 1], in_=b2[128 * ct : 128 * ct + 128, :])
                dma.dma_start(out=w1_sb[ct], in_=w1t[128 * ct : 128 * ct + 128, :])
            w2_sb = cp.tile([16, C], fp32)
            dma.dma_start(out=w2_sb, in_=w2t[:, :])

            x_sb = [sp.tile([128, SLAB, W], bf16, tag=f"x{ct}", name=f"x{ct}") for ct in range(2)]
            dma.dma_start(out=x_sb[0], in_=xs[0:128, :, :])
            nc.scalar.dma_start(out=x_sb[1], in_=xs[128:256, :, :])

            msk_sb = cp.tile([128, 2], fp32)
            nc.gpsimd.dma_start(out=msk_sb, in_=msk[:, :])
            dw_sb = [cp.tile([128, C], bf16, tag=f"dw_{ct}", name=f"dw_{ct}") for ct in range(2)]
            for ct in range(2):
                nc.gpsimd.dma_start(out=dw_sb[ct], in_=dwt[128 * ct : 128 * ct + 128, :])

            # ---------- constants ----------
            ident = cp.tile([128, 128], bf16)
            make_identity(nc, ident)
            e_all = cp.tile([128, 9, 9], bf16)
            nc.gpsimd.memset(e_all, 0.0)
            for d in range(9):
                nc.gpsimd.memset(e_all[:, d, d : d + 1], 1.0)
            ones_99 = cp.tile([9, 9], bf16)
            nc.gpsimd.memset(ones_99, 1.0)
            eps_sb = cp.tile([128, 1], fp32)
            nc.gpsimd.memset(eps_sb, 1e-5)
            dm1 = cp.tile([1, 1], fp32)
            dm2 = cp.tile([1, 1], fp32)
            nc.vector.memset(dm1, 0.0)
            nc.vector.memset(dm2, 0.0)

            # ---------- SE gate from host-shipped channel mean ----------
            with tc.tile_pool(name="ps_se", bufs=1, space="PSUM") as pse:
                h_ps = pse.tile([16, 1], fp32)
                for ct in range(2):
                    nc.tensor.matmul(
                        h_ps, w1_sb[ct], mn_sb[:, ct : ct + 1],
                        start=(ct == 0), stop=(ct == 1),
                    )
                h_sb = sp.tile([16, 1], fp32)
                # relu on DVE (avoids burning a ScalarE table slot on Relu)
                nc.vector.tensor_scalar(
                    out=h_sb, in0=h_ps, scalar1=b1_sb[:, 0:1], scalar2=0.0,
                    op0=Alu.add, op1=Alu.max,
                )
                y_ps = pse.tile([128, 2], fp32)
                y_sb = sp.tile([128, 2], fp32, tag="ygate")
                for ct in range(2):
                    nc.tensor.matmul(
                        y_ps[:, ct : ct + 1], w2_sb[:, 128 * ct : 128 * ct + 128], h_sb,
                        start=True, stop=True,
                    )
                    # first ScalarE op in the program: loads the sigmoid table
                    # set while the x slabs are still streaming in
                    nc.scalar.activation(out=y_sb[:, ct : ct + 1], in_=y_ps[:, ct : ct + 1],
                                         func=AF.Sigmoid, bias=b2_sb[:, ct : ct + 1], scale=1.0)

            # ---------- FF and S maps (bf16, zero-padded 14x100 layout) ------
            ff_sb = [sp.tile([128, SLAB, WP], bf16, tag=f"ff{ct}", name=f"ff{ct}") for ct in range(2)]
            s_sb = [sp.tile([128, SLAB, WP], bf16, tag=f"s{ct}", name=f"s{ct}") for ct in range(2)]
            ff2_sb = [sp.tile([128, SLAB, WP], bf16, tag=f"ff2{ct}", name=f"ff2{ct}") for ct in range(2)]
            s2_sb = [sp.tile([128, SLAB, WP], bf16, tag=f"s2{ct}", name=f"s2{ct}") for ct in range(2)]
            for ct in range(2):
                nc.vector.memset(s2_sb[ct][:, :, 2:3], 0.0)
                nc.vector.memset(s2_sb[ct][:, :, 99:100], 0.0)
                nc.vector.memset(ff2_sb[ct][:, :, 2:3], 0.0)
                nc.vector.memset(ff2_sb[ct][:, :, 99:100], 0.0)
                nc.vector.tensor_scalar(
                    out=ff_sb[ct][:, :, 2:98], in0=x_sb[ct],
                    scalar1=y_sb[:, ct : ct + 1], scalar2=None, op0=Alu.mult,
                )
                nc.scalar.activation(
                    out=s_sb[ct][:, :, 2:98], in_=x_sb[ct],
                    func=AF.Sigmoid, scale=y_sb[:, ct : ct + 1],
                )
                nc.vector.tensor_scalar(
                    out=s_sb[ct][:, 0, 2:98], in0=s_sb[ct][:, 0, 2:98],
                    scalar1=msk_sb[:, 0:1], scalar2=None, op0=Alu.mult,
                )
                nc.vector.tensor_scalar(
                    out=s_sb[ct][:, 13, 2:98], in0=s_sb[ct][:, 13, 2:98],
                    scalar1=msk_sb[:, 1:2], scalar2=None, op0=Alu.mult,
                )
                nc.vector.tensor_copy(out=s2_sb[ct][:, :, 3:99], in_=s_sb[ct][:, :, 2:98])
                nc.vector.tensor_copy(out=ff2_sb[ct][:, :, 3:99], in_=ff_sb[ct][:, :, 2:98])

            # preload the exp table set while phase 2 runs; the input is a
            # slice of the ct1 S map purely to pin this op's position in the
            # ScalarE queue (after the S sigmoids, before the real exp)
            nc.scalar.activation(out=dm1, in_=s_sb[1][0:1, 0, 2:3], func=AF.Exp)

            def sview(ct, di, dj):
                if dj == 0:
                    return s_sb[ct][:, 1 + di : 13 + di, 2:98]
                return s2_sb[ct][:, 1 + di : 13 + di, 3 + dj : 99 + dj]

            def ffview(ct, di, dj):
                if dj == 0:
                    return ff_sb[ct][:, 1 + di : 13 + di, 2:98]
                return ff2_sb[ct][:, 1 + di : 13 + di, 3 + dj : 99 + dj]

            # ---------- phase 2: correlation  A[d, p] = sum_c S*S_d ----------
            with tc.tile_pool(name="ps_corr", bufs=1, space="PSUM") as pc:
                A_ps = pc.tile([9, P], fp32)
                for d, (di, dj) in enumerate(SHIFTS):
                    for ct in range(2):
                        # all products on DVE: GpSimd is ~3x slower per op and
                        # pays a ~4us program-load penalty on its first use,
                        # which made it the phase-2 tail when offloaded here
                        veng = nc.vector
                        prod = wp.tile([128, P], bf16, tag="prod")
                        pv = prod.rearrange("c (r w) -> c r w", w=W)
                        veng.tensor_tensor(
                            out=pv,
                            in0=s_sb[ct][:, 1:13, 2:98],
                            in1=sview(ct, di, dj),
                            op=Alu.mult,
                        )
                        for (o, n) in CHUNKS:
                            nc.tensor.matmul(
                                A_ps[:, o : o + n], e_all[:, d, :], prod[:, o : o + n],
                                start=(d == 0 and ct == 0), stop=(d == 8 and ct == 1),
                            )
                # softmax numerator: exp with the folded 1/C mean.  The
                # 1/denominator is deferred to after the phase-3 psum
                # accumulation so the row broadcasts start immediately.
                exp_sb = sp.tile([9, P], bf16, tag="exps")
                nc.scalar.activation(out=exp_sb, in_=A_ps, func=AF.Exp, scale=1.0 / C)
                dma.dma_start(out=a_dram[:, :], in_=exp_sb)
                den_ps = pc.tile([9, P], fp32)
                for (o, n) in CHUNKS:
                    nc.tensor.matmul(den_ps[:, o : o + n], ones_99, exp_sb[:, o : o + n],
                                     start=True, stop=True)
                rec_sb = sp.tile([9, P], fp32, tag="recs")
                nc.vector.reciprocal_approx_fast(out=rec_sb, in_=den_ps)

            # ---------- replicate exp rows + recip across 128 partitions ----
            rep_sb = [sp.tile([128, P], bf16, tag=f"rep{d}", name=f"rep{d}") for d in range(9)]
            for d in range(9):
                eng = dma if d % 2 == 0 else nc.scalar
                eng.dma_start(
                    out=rep_sb[d],
                    in_=a_dram[d, :].partition_broadcast(128),
                )
            rec_row = sp.tile([1, P], fp32, tag="recrow")
            nc.vector.tensor_copy(out=rec_row, in_=rec_sb[0:1, :])
            nc.scalar.dma_start(out=r_dram[:], in_=rec_row)
            rec_rep = sp.tile([128, P], fp32, tag="recrep")
            nc.scalar.dma_start(out=rec_rep, in_=r_dram[:].partition_broadcast(128))
            # preload the sqrt/prelu table set during phase 3 / a2a; input
            # slice of exp_sb pins it after the real exp in the ScalarE queue
            nc.scalar.activation(out=dm2, in_=exp_sb[0:1, 0:1], func=AF.Sqrt)

            # ---------- phase 3: weighted sum  out[c,p] = sum_d a_d * FF_d ---
            with tc.tile_pool(name="ps_acc", bufs=1, space="PSUM") as pa:
                for ct in range(2):
                    acc = pa.tile([128, P], fp32, tag=f"acc{ct}")
                    for d, (di, dj) in enumerate(SHIFTS):
                        veng = nc.gpsimd if d == 8 else nc.vector
                        prod = wp.tile([128, P], bf16, tag="prod")
                        pv = prod.rearrange("c (r w) -> c r w", w=W)
                        veng.tensor_tensor(
                            out=pv,
                            in0=ffview(ct, di, dj),
                            in1=rep_sb[d].rearrange("c (r w) -> c r w", w=W),
                            op=Alu.mult,
                        )
                        for (o, n) in CHUNKS:
                            nc.tensor.matmul(
                                acc[:, o : o + n], ident, prod[:, o : o + n],
                                start=(d == 0), stop=(d == 8),
                            )
                    # deferred softmax denominator: oat = acc * (1/den)
                    oat = sp.tile([128, P], bf16, tag=f"oat{ct}", name=f"oat{ct}")
                    nc.vector.tensor_tensor(out=oat, in0=acc, in1=rec_rep, op=Alu.mult)
                    eng = dma if ct == 0 else nc.scalar
                    eng.dma_start(out=a2a_in[4 * ct : 4 * ct + 4, :, :], in_=oat)

            # ---------- AllToAll ----------
            nc.gpsimd.collective_compute(
                "AllToAll", Alu.bypass, replica_groups=GROUPS,
                ins=[a2a_in[:, :, :]], outs=[a2a_out[:, :, :]],
            )
            # prefetch the first down-matmul weights into the PE while the
            # a2a is in flight, so the first post-a2a matmul skips LDWEIGHTS
            nc.tensor.ldweights(dw_sb[0][:, 0:128])

            # ---------- down matmul: strided rhs de-scramble on read --------
            # rhs[kt][(j,b), s, t] = a2a_out[4kt+j, s, 36b+t]
            rhs_sb = [sp.tile([128, 32, 36], bf16, tag=f"rhs{kt}", name=f"rhs{kt}") for kt in range(2)]
            rhs_engs = [dma, nc.scalar, nc.gpsimd]
            a2a_bv = a2a_out.rearrange("j s (b t) -> j b s t", t=36)
            for kt in range(2):
                for jj in range(4):
                    rhs_engs[(4 * kt + jj) % 3].dma_start(
                        out=rhs_sb[kt][32 * jj : 32 * jj + 32, :, :],
                        in_=a2a_bv[4 * kt + jj],
                    )

            stat_sb = sp.tile([128, 4], fp32, tag="stat")
            sq_scr = wp.tile([128, P], fp32, tag="sqscr")
            zo_sb = [sp.tile([128, P], bf16, tag=f"zo{mt}", name=f"zo{mt}") for mt in range(2)]
            with tc.tile_pool(name="ps_z", bufs=1, space="PSUM") as pz:
                z_ps = [pz.tile([128, P], fp32, tag=f"z{mt}", name=f"z{mt}") for mt in range(2)]
                # kt-outer: all kt0 passes run while the strided kt1 rhs
                # loads are still landing; weights reload only 4x total
                for kt in range(2):
                    for mt in range(2):
                        for (o, n) in CHUNKS:
                            nc.tensor.matmul(
                                z_ps[mt][:, o : o + n],
                                dw_sb[kt][:, 128 * mt : 128 * mt + 128],
                                rhs_sb[kt].rearrange("c s t -> c (s t)")[:, o : o + n],
                                start=(kt == 0), stop=(kt == 1),
                            )
                for mt in range(2):
                    nc.vector.tensor_reduce(
                        out=stat_sb[:, mt : mt + 1], in_=z_ps[mt],
                        axis=mybir.AxisListType.X, op=Alu.add,
                    )
                    nc.scalar.activation(
                        out=sq_scr, in_=z_ps[mt], func=AF.Square,
                        accum_out=stat_sb[:, 2 + mt : 3 + mt],
                    )
                nc.gpsimd.dma_start(out=st_part[:, :], in_=stat_sb)
                nc.gpsimd.collective_compute(
                    "AllReduce", Alu.add, replica_groups=GROUPS,
                    ins=[st_part[:, :]], outs=[st_sum[:, :]],
                )
                gl_sb = sp.tile([128, 4], fp32, tag="glstat")
                nc.gpsimd.dma_start(out=gl_sb, in_=st_sum[:, :])

                # mu = sum/HW ; var = sumsq/HW - mu^2 ; inv = rsqrt(var+eps)
                ins_sb = sp.tile([128, 8], fp32, tag="instat")
                g4 = ins_sb[:, 0:4]         # [mu0, mu1, e20, e21]
                mu2 = ins_sb[:, 0:2]
                e22 = ins_sb[:, 2:4]
                inv2 = ins_sb[:, 4:6]
                nmi2 = ins_sb[:, 6:8]
                nc.vector.tensor_scalar(out=g4, in0=gl_sb[:, 0:4],
                                        scalar1=1.0 / HW, scalar2=None, op0=Alu.mult)
                nc.vector.tensor_tensor(out=inv2, in0=mu2, in1=mu2, op=Alu.mult)
                nc.vector.tensor_tensor(out=e22, in0=e22, in1=inv2, op=Alu.subtract)
                nc.scalar.activation(out=e22, in_=e22, func=AF.Sqrt, bias=eps_sb, scale=1.0)
                nc.vector.reciprocal(out=inv2, in_=e22)
                nc.vector.scalar_tensor_tensor(out=nmi2, in0=mu2, scalar=-1.0,
                                               in1=inv2, op0=Alu.mult, op1=Alu.mult)
                # LeakyReLU((z - mu) * inv): mt0 fused on ScalarE as
                # prelu(z*inv + (-mu*inv), alpha=0.2); mt1 on DVE as
                # w = z*inv + nmi ; out = max(w, 0.2*w) — the two halves run
                # on different engines concurrently.
                nc.scalar.activation(
                    out=zo_sb[0], in_=z_ps[0], func=AF.Prelu,
                    bias=ins_sb[:, 6:7], scale=ins_sb[:, 4:5], alpha=0.2,
                )
                dma.dma_start(
                    out=outp[0:128, :, :],
                    in_=zo_sb[0].rearrange("c (s t) -> c s t", t=36),
                )
                w_sb = wp.tile([128, P], fp32, tag="wnorm")
                nc.vector.tensor_scalar(
                    out=w_sb, in0=z_ps[1], scalar1=ins_sb[:, 5:6],
                    scalar2=ins_sb[:, 7:8], op0=Alu.mult, op1=Alu.add,
                )
                nc.vector.scalar_tensor_tensor(
                    out=zo_sb[1], in0=w_sb, scalar=0.2, in1=w_sb,
                    op0=Alu.mult, op1=Alu.max,
                )
                nc.scalar.dma_start(
                    out=outp[128:256, :, :],
                    in_=zo_sb[1].rearrange("c (s t) -> c s t", t=36),
                )
    nc.compile()
    return nc


def _get_nc():
    if "nc" not in _cache:
        _cache["nc"] = _build()
    return _cache["nc"]


def _shard_inputs(x, se_w1, se_b1, se_w2, se_b2, down_w):
    import ml_dtypes

    x = np.ascontiguousarray(np.asarray(x, np.float32))[0]          # (C, H, W)
    mean = x.mean(axis=(1, 2))                                      # (C,)
    mn = np.ascontiguousarray(mean.reshape(2, 128).T).astype(np.float32)
    w1t = np.ascontiguousarray(np.asarray(se_w1, np.float32).T)     # (C, 16)
    b1 = np.ascontiguousarray(np.asarray(se_b1, np.float32)[:, None])
    w2t = np.ascontiguousarray(np.asarray(se_w2, np.float32).T)     # (16, C)
    b2 = np.ascontiguousarray(np.asarray(se_b2, np.float32)[:, None])
    dwt = np.ascontiguousarray(
        np.asarray(down_w, np.float32).T.astype(ml_dtypes.bfloat16)
    )                                                               # (C, C) bf16

    in_maps = []
    for k in range(M):
        slab = np.zeros((C, SLAB, W), ml_dtypes.bfloat16)
        lo, hi = RPC * k - 1, RPC * k + RPC + 1
        clo, chi = max(lo, 0), min(hi, H)
        slab[:, clo - lo : clo - lo + (chi - clo), :] = x[:, clo:chi, :].astype(
            ml_dtypes.bfloat16
        )
        msk = np.ones((128, 2), np.float32)
        if k == 0:
            msk[:, 0] = 0.0
        if k == M - 1:
            msk[:, 1] = 0.0
        in_maps.append({
            "xs": slab, "mn": mn, "msk": msk, "w1t": w1t, "b1": b1,
            "w2t": w2t, "b2": b2, "dwt": dwt,
        })
    return in_maps


def _gather(results):
    R = np.stack([np.asarray(r["out"]).astype(np.float32) for r in results])
    return np.ascontiguousarray(
        R.transpose(1, 3, 0, 2).reshape(1, C, H, W).astype(np.float32)
    )


def kernel(x, se_w1, se_b1, se_w2, se_b2, down_w, _trace=False):
    from concourse.bass_utils import run_bass_kernel_spmd

    nc = _get_nc()
    in_maps = _shard_inputs(x, se_w1, se_b1, se_w2, se_b2, down_w)
    res = run_bass_kernel_spmd(nc, in_maps, core_ids=list(range(M)), trace=_trace)
    out = _gather(res.results)
    if _trace:
        kernel.last_results = res
    return out
